# revision 26
# baseline (speedup 1.0000x reference)
"""Trainium2 Bass kernel for CrossAttention (nn_CrossAttention_27255862460837).

Shards the (B, N) token axis of x across 8 NeuronCores (512 tokens per batch
per core); context / weights are replicated.  Per core:
    qT = (Wq^T @ x^T) * SCALE        (fp32r matmuls, N=512 -> full rate)
    kT_bh, v_b from context          (bf16 matmuls, one-time setup)
    sim = q @ k^T                    (bf16, psum fp32) -> rear_sim output
    attn = softmax(sim) (no max-sub; logits are tiny), reweight folded into v
    out = (attn @ v') @ Wo + bo      (bf16 matmuls, fp32 accum)

Heads are packed 2 per 128-partition group (rows 0-39 and 64-103, gaps
zeroed).  All matmul operands start at partition 0 - nonzero base partitions
crash this hardware - so per-head products are separated by zero-masking:
  sim:  one K=128 matmul per group against kT [128, 2*77] whose per-head
        column blocks are zero outside that head's rows.
  av:   two accumulating matmuls per group with v padded to [77, 128]
        (head columns at 0-39 / 64-103, zeros elsewhere).
"""

import os
import sys

sys.path.insert(0, "/opt/trn_rl_repo")

import numpy as np

import concourse.bass as bass
import concourse.mybir as mybir
import concourse.tile as tile
import bass_rust
from concourse.masks import make_identity
from concourse.bass_utils import run_bass_kernel_spmd

# ---------------------------------------------------------------------------
# Workaround: walrus codegen rejects >1 sync wait on one instruction, but
# Tile's scheduler (and its final drain) can attach more.  Post-pass: move
# excess waits onto EventSemaphore instructions inserted just before, on the
# same engine.
# ---------------------------------------------------------------------------


def _split_excess_waits(nc: bass.Bass, max_waits: int = 1) -> None:
    for f in nc.m.functions:
        for bb in f.blocks:
            out = []
            changed = False
            for inst in bb.instructions:
                si = inst.sync_info
                waits = list(si.on_wait) if si is not None else []
                if len(waits) > max_waits:
                    changed = True
                    extra = waits[:-max_waits]
                    for i in range(0, len(extra), max_waits):
                        ev = mybir.InstEventSemaphore(
                            name=nc.get_next_instruction_name(), ins=[], outs=[]
                        )
                        ev.engine = inst.engine
                        ev.sync_info = bass_rust.SyncInfo(
                            on_wait=extra[i : i + max_waits], on_update=[]
                        )
                        out.append(ev)
                    inst.sync_info = bass_rust.SyncInfo(
                        on_wait=waits[-max_waits:], on_update=list(si.on_update)
                    )
                out.append(inst)
            if changed:
                bb.instructions = out


# ---------------------------------------------------------------------------

P = 128
B, N, QD = 6, 4096, 320
CN, CD = 77, 768
H, DH = 8, 40
N_CORES = 8
NPC = N // N_CORES  # 512 tokens per batch per core
NT = NPC // P  # 4 token tiles of 128
NG = 4  # head-pair groups; head h -> group h//2, col offset (h%2)*64
CN2 = 2 * CN  # two per-head column blocks in the fused sim matmul
SCALE = DH**-0.5
REWEIGHTS = [-2.0, 1.0, 5.0]  # applied to v row TOKEN_IDX for b = 3,4,5
TOKEN_IDX = 2

FP32 = mybir.dt.float32
F32R = mybir.dt.float32r
BF16 = mybir.dt.bfloat16
EXP = mybir.ActivationFunctionType.Exp
AX = mybir.AxisListType.X
MUL = mybir.AluOpType.mult
ADD = mybir.AluOpType.add

KC = [128, 128, 64]  # K chunks of QD=320


def build_nc() -> bass.Bass:
    nc = bass.Bass()
    x_d = nc.declare_dram_parameter("x", [B, NPC, QD], FP32, isOutput=False)
    ctx_d = nc.declare_dram_parameter("context", [B, CN, CD], FP32, isOutput=False)
    wq_d = nc.declare_dram_parameter("Wq", [QD, QD], FP32, isOutput=False)
    wk_d = nc.declare_dram_parameter("Wk", [CD, QD], FP32, isOutput=False)
    wv_d = nc.declare_dram_parameter("Wv", [CD, QD], FP32, isOutput=False)
    wo_d = nc.declare_dram_parameter("Wo", [QD, QD], FP32, isOutput=False)
    bo_d = nc.declare_dram_parameter("bo", [QD], FP32, isOutput=False)
    # host-built reweight columns: rw[j, b] = w_b[j] (ones except b>=3, j=2)
    rw_d = nc.declare_dram_parameter("rw", [CN, B], FP32, isOutput=False)
    out_d = nc.declare_dram_parameter("out", [B, NPC, QD], FP32, isOutput=True)
    rear_d = nc.declare_dram_parameter("rear", [B, H, NPC, CN], FP32, isOutput=True)

    with tile.TileContext(nc) as tc:
        with (
            tc.tile_pool(name="consts", bufs=1) as consts,
            tc.tile_pool(name="wstage", bufs=1) as wstage,
            tc.tile_pool(name="kvstage", bufs=2) as kvstage,
            tc.tile_pool(name="xload", bufs=2) as xlp,
            tc.tile_pool(name="xt", bufs=2) as xtp,
            tc.tile_pool(name="qt", bufs=2) as qtp,
            tc.tile_pool(name="soft", bufs=4) as softp,
            tc.tile_pool(name="att", bufs=4) as attp,
            tc.tile_pool(name="ot", bufs=2) as otp,
            tc.tile_pool(name="outs", bufs=2) as outp,
            tc.tile_pool(name="ps_sim", bufs=2, space="PSUM") as ps_sim,
            tc.tile_pool(name="ps_tr", bufs=2, space="PSUM") as ps_tr,
            tc.tile_pool(name="ps_o", bufs=1, space="PSUM") as ps_o,
            tc.tile_pool(name="ps_f", bufs=1, space="PSUM") as ps_f,
            tc.tile_pool(name="ps_q", bufs=1, space="PSUM") as ps_q,
            tc.tile_pool(name="ps_aux", bufs=1, space="PSUM") as ps_aux,
        ):
            # ---------------- one-time setup ----------------
            id_f32 = consts.tile([P, P], FP32)
            make_identity(nc, id_f32)
            id_bf = consts.tile([P, P], BF16)
            nc.vector.tensor_copy(id_bf, id_f32)

            zeros_col = consts.tile([P, 1], FP32)
            nc.any.memset(zeros_col, 0.0)

            # Wq natural fp32, then padded head-pair layout [128, 3c, 4g, 128]
            wq_f32 = wstage.tile([P, 3, QD], FP32, tag="wqnat")
            nc.any.memset(wq_f32, 0.0)  # chunk 2 rows 64-127 never loaded
            for c in range(3):
                nc.sync.dma_start(
                    wq_f32[: KC[c], c, :], wq_d[c * 128 : c * 128 + KC[c], :]
                )
            wq_pad = consts.tile([P, 3, NG, P], F32R)
            # zero-fill via DVE copy: birverifier requires f32r-consumed
            # tensors be produced by rounding ops (memset is not one)
            nc.vector.tensor_copy(wq_pad, zeros_col.broadcast_to([P, 3, NG, P]))
            for g in range(NG):
                for h2 in range(2):
                    nc.vector.tensor_copy(
                        wq_pad[:, :, g, h2 * 64 : h2 * 64 + DH],
                        wq_f32[:, :, (2 * g + h2) * DH : (2 * g + h2 + 1) * DH],
                    )
            # Wk -> bf16, separate lo/hi padded tiles [128, 6c, 4g, 128]:
            # _lo holds only each pair's even head (cols 0-39), _hi only the
            # odd head (cols 64-103); all other columns zero.  The resulting
            # kT blocks are zero outside that head's rows, which masks the
            # cross-head terms in the fused K=128 sim matmul.
            wk_stage = wstage.tile([P, 6, QD], FP32, tag="wstage")
            nc.sync.dma_start(wk_stage, wk_d[:, :].rearrange("(c p) m -> p c m", p=P))
            wk_nat = wstage.tile([P, 6, QD], BF16, tag="wknat")
            nc.vector.tensor_copy(wk_nat, wk_stage)
            wk_lo = consts.tile([P, 6, NG, P], BF16)
            wk_hi = consts.tile([P, 6, NG, P], BF16)
            nc.any.memset(wk_lo, 0.0)
            nc.any.memset(wk_hi, 0.0)
            for g in range(NG):
                nc.vector.tensor_copy(
                    wk_lo[:, :, g, 0:DH],
                    wk_nat[:, :, (2 * g) * DH : (2 * g + 1) * DH],
                )
                nc.vector.tensor_copy(
                    wk_hi[:, :, g, 64 : 64 + DH],
                    wk_nat[:, :, (2 * g + 1) * DH : (2 * g + 2) * DH],
                )
            wv_stage = wstage.tile([P, 6, QD], FP32, tag="wstage")
            nc.sync.dma_start(wv_stage, wv_d[:, :].rearrange("(c p) m -> p c m", p=P))
            wv_bf = consts.tile([P, 6, QD], BF16)
            nc.vector.tensor_copy(wv_bf, wv_stage)
            # Wo padded rows: group g holds Wo rows (2g)*40..(2g+2)*40 at
            # partitions 0-39 and 64-103, zeros elsewhere.
            wo_stage = wstage.tile([P, NG, QD], FP32, tag="wostage")
            nc.any.memset(wo_stage, 0.0)
            for g in range(NG):
                for h2 in range(2):
                    r = (2 * g + h2) * DH
                    nc.sync.dma_start(
                        wo_stage[h2 * 64 : h2 * 64 + DH, g, :],
                        wo_d[r : r + DH, :],
                    )
            wo_pad = consts.tile([P, NG, QD], BF16)
            nc.vector.tensor_copy(wo_pad, wo_stage)
            # bo broadcast to [128, 320] via ones outer product
            bo_row = consts.tile([1, QD], FP32)
            nc.sync.dma_start(bo_row, bo_d[None, :])
            ones_row = consts.tile([1, P], FP32)
            nc.any.memset(ones_row, 1.0)
            bo_ps = ps_f.tile([P, QD], FP32, tag="psf")
            nc.tensor.matmul(bo_ps, ones_row, bo_row, start=True, stop=True)
            bo_bcast = consts.tile([P, QD], FP32)
            nc.vector.tensor_copy(bo_bcast, bo_ps)

            rw_sb = consts.tile([CN, B], FP32)
            nc.sync.dma_start(rw_sb, rw_d[:, :])

            # context -> bf16, all batches  [77, B, 768]
            ctx_stage = wstage.tile([CN, B, CD], FP32, tag="ctxstage")
            nc.sync.dma_start(ctx_stage, ctx_d[:, :, :].rearrange("b j d -> j b d"))
            ctx_bf = wstage.tile([CN, B, CD], BF16, tag="ctxbf")
            nc.vector.tensor_copy(ctx_bf, ctx_stage)

            # per-batch: ctxT, then kT pair-blocks and padded v
            # kT_bf[:, b, g, 0:77] = kT of even head (zero outside its rows),
            # kT_bf[:, b, g, 77:154] = odd head.
            kT_bf = consts.tile([P, B, NG, CN2], BF16)
            # v padded per head parity: v_lo holds only even heads' columns
            # (0-39), v_hi only odd heads' (64-103); zeros elsewhere so the
            # two accumulating av matmuls don't cross-pollute row ranges.
            v_lo = consts.tile([CN, B, NG, P], BF16)
            v_hi = consts.tile([CN, B, NG, P], BF16)
            nc.any.memset(v_lo, 0.0)
            nc.any.memset(v_hi, 0.0)
            for b in range(B):
                # inner dim padded to 78: bf16 PSUM slices must be 4B aligned
                ctxT_ps = ps_aux.tile([P, 6, CN + 1], BF16, tag="psaux")
                for c in range(6):
                    nc.tensor.transpose(
                        ctxT_ps[:, c, :CN],
                        ctx_bf[:, b, c * 128 : (c + 1) * 128],
                        id_bf[:CN, :CN],
                    )
                ctxT_bf = kvstage.tile([P, 6, CN], BF16, tag="ctxT")
                nc.vector.tensor_copy(ctxT_bf, ctxT_ps[:, :, :CN])

                for g in range(NG):
                    kps = ps_aux.tile([P, CN2], FP32, tag="psaux")
                    for c in range(6):
                        nc.tensor.matmul(
                            kps[:, 0:CN],
                            wk_lo[:, c, g, :],
                            ctxT_bf[:, c, :],
                            start=(c == 0),
                            stop=(c == 5),
                        )
                    for c in range(6):
                        nc.tensor.matmul(
                            kps[:, CN:CN2],
                            wk_hi[:, c, g, :],
                            ctxT_bf[:, c, :],
                            start=(c == 0),
                            stop=(c == 5),
                        )
                    nc.vector.tensor_copy(kT_bf[:, b, g, :], kps)

                vps = ps_aux.tile([CN, QD], FP32, tag="psaux")
                for c in range(6):
                    nc.tensor.matmul(
                        vps,
                        ctxT_bf[:, c, :],
                        wv_bf[:, c, :],
                        start=(c == 0),
                        stop=(c == 5),
                    )
                # cross_attention_control reweight folded into v rows; also
                # scatter heads into zero-padded per-group column blocks
                for h in range(H):
                    g, o = h // 2, (h % 2) * 64
                    dst = v_lo if h % 2 == 0 else v_hi
                    nc.vector.tensor_scalar_mul(
                        dst[:, b, g, o : o + DH],
                        vps[:, h * DH : (h + 1) * DH],
                        rw_sb[:, b : b + 1],
                    )

            # ---------------- main loop ----------------
            for b in range(B):
                x_sb = xlp.tile([P, NT, QD], FP32, tag="x")
                nc.sync.dma_start(x_sb, x_d[b].rearrange("(t p) m -> p t m", p=P))
                # transpose x (fp32): xT chunks [kc, t, 128]
                xT_f32 = xtp.tile([P, 3, NT, P], F32R, tag="xT")
                for c in range(3):
                    xps = ps_aux.tile([P, NT, P], FP32, tag="psaux")
                    for t in range(NT):
                        nc.tensor.transpose(
                            xps[: KC[c], t, :],
                            x_sb[:, t, c * 128 : c * 128 + KC[c]],
                            id_f32,
                        )
                    nc.vector.tensor_copy(xT_f32[: KC[c], c], xps[: KC[c]])

                # qT = Wq^T x^T (f32r), scaled by SCALE on copy -> bf16
                qT_bf = qtp.tile([P, NG, NPC], BF16, tag="qT")
                for g in range(NG):
                    qps = ps_q.tile([P, NPC], FP32, tag="psq")
                    for c in range(3):
                        nc.tensor.matmul(
                            qps,
                            wq_pad[: KC[c], c, g, :],
                            xT_f32[: KC[c], c].rearrange("p t n -> p (t n)"),
                            start=(c == 0),
                            stop=(c == 2),
                        )
                    nc.vector.tensor_scalar_mul(qT_bf[:, g, :], qps, SCALE)

                for t in range(NT):
                    sums = softp.tile([P, H], FP32, tag="sums")
                    recip = softp.tile([P, H], FP32, tag="recip")
                    exp_tiles = []
                    for half in range(2):
                        # [128, 2, 154]: same memory layout as [128, 4, 77] -
                        # head h at free offset (h%4)*77
                        pssim = ps_sim.tile([P, 2, CN2], FP32, tag="pssim")
                        for gg in range(2):
                            g = half * 2 + gg
                            nc.tensor.matmul(
                                pssim[:, gg, :],
                                qT_bf[:, g, t * P : (t + 1) * P],
                                kT_bf[:, b, g, :],
                                start=True,
                                stop=True,
                            )
                        # rear_sim out: already scaled (SCALE folded into qT)
                        sim_sb = softp.tile([P, 4, CN], FP32, tag="simsb")
                        nc.scalar.copy(sim_sb, pssim.rearrange("p g j -> p (g j)"))
                        nc.sync.dma_start(
                            rear_d[
                                b, half * 4 : (half + 1) * 4, t * P : (t + 1) * P, :
                            ].rearrange("h n j -> n h j"),
                            sim_sb,
                        )
                        exp_bf = softp.tile([P, 4, CN], BF16, tag="expb")
                        nc.scalar.activation(
                            exp_bf, pssim.rearrange("p g j -> p (g j)"), EXP
                        )
                        nc.vector.reduce_sum(
                            sums[:, half * 4 : (half + 1) * 4], exp_bf, axis=AX
                        )
                        exp_tiles.append(exp_bf)
                    nc.vector.reciprocal(recip, sums)
                    attnT_tiles = []
                    for half in range(2):
                        attn_bf = attp.tile([P, 4, CN], BF16, tag="attnb")
                        nc.vector.tensor_tensor(
                            out=attn_bf,
                            in0=exp_tiles[half],
                            in1=recip[:, half * 4 : (half + 1) * 4].broadcast_to(
                                [P, 4, CN]
                            ),
                            op=MUL,
                        )
                        pstr = ps_tr.tile([CN, 4, P], BF16, tag="pstr")
                        for hh in range(4):
                            nc.tensor.transpose(
                                pstr[:, hh, :], attn_bf[:, hh, :], id_bf
                            )
                        attnT_bf = attp.tile([CN, 4, P], BF16, tag="attnT")
                        nc.scalar.copy(attnT_bf, pstr)
                        attnT_tiles.append(attnT_bf)

                    pso = ps_o.tile([P, NG, P], FP32, tag="pso")
                    for h in range(H):
                        g = h // 2
                        vsrc = v_lo if h % 2 == 0 else v_hi
                        nc.tensor.matmul(
                            pso[:, g, :],
                            vsrc[:, b, g, :],
                            attnT_tiles[h // 4][:, h % 4, :],
                            start=(h % 2 == 0),
                            stop=(h % 2 == 1),
                        )
                    oT_bf = otp.tile([P, NG, P], BF16, tag="oT")
                    nc.vector.tensor_copy(oT_bf, pso)
                    psf = ps_f.tile([P, QD], FP32, tag="psf")
                    for g in range(NG):
                        nc.tensor.matmul(
                            psf,
                            oT_bf[:, g, :],
                            wo_pad[:, g, :],
                            start=(g == 0),
                            stop=(g == 3),
                        )
                    out_sb = outp.tile([P, QD], FP32, tag="outsb")
                    nc.vector.tensor_tensor(out=out_sb, in0=psf, in1=bo_bcast, op=ADD)
                    nc.sync.dma_start(out_d[b, t * P : (t + 1) * P, :], out_sb)

    _split_excess_waits(nc)
    return nc


_NC_CACHE = None


def _get_nc():
    global _NC_CACHE
    if _NC_CACHE is None:
        _NC_CACHE = build_nc()
    return _NC_CACHE


def kernel(**inputs) -> tuple[np.ndarray, np.ndarray]:
    x = np.ascontiguousarray(np.asarray(inputs["x"], dtype=np.float32))
    context = np.ascontiguousarray(np.asarray(inputs["context"], dtype=np.float32))
    Wq = np.ascontiguousarray(np.asarray(inputs["Wq"], dtype=np.float32))
    Wk = np.ascontiguousarray(np.asarray(inputs["Wk"], dtype=np.float32))
    Wv = np.ascontiguousarray(np.asarray(inputs["Wv"], dtype=np.float32))
    Wo = np.ascontiguousarray(np.asarray(inputs["Wo"], dtype=np.float32))
    bo = np.ascontiguousarray(np.asarray(inputs["bo"], dtype=np.float32))

    rw = np.ones((CN, B), dtype=np.float32)
    for i, w in enumerate(REWEIGHTS):
        rw[TOKEN_IDX, 3 + i] = w

    nc = _get_nc()
    in_maps = []
    for c in range(N_CORES):
        in_maps.append(
            {
                "x": np.ascontiguousarray(x[:, c * NPC : (c + 1) * NPC, :]),
                "context": context,
                "Wq": Wq,
                "Wk": Wk,
                "Wv": Wv,
                "Wo": Wo,
                "bo": bo,
                "rw": rw,
            }
        )
    res = run_bass_kernel_spmd(nc, in_maps, list(range(N_CORES)))
    out = np.concatenate([res.results[c]["out"] for c in range(N_CORES)], axis=1)
    rear = np.concatenate([res.results[c]["rear"] for c in range(N_CORES)], axis=2)
    return out, rear
